# revision 1
# baseline (speedup 1.0000x reference)
"""MoE-routing actor kernel for 8 Trainium2 NeuronCores.

Strategy (pure data parallel, expert-sorted):
  - Host: for each expert m, deal its rows round-robin to the 8 cores so all
    cores get near-identical per-expert counts and can share ONE SPMD graph.
  - Host packs each core's rows grouped by expert (padded to 128-row tiles),
    pre-transposed to [32, T*128] so the device needs no transposes.
  - Bias + routing bias + mask are folded into effective expert weights:
    an extra all-ones activation row (row 34) times a weight row holding
    bout (or -1e9 on masked columns, whose weight columns are zeroed).
  - Device per 128-row tile: fc1 matmul (stationary W1T_aug, K=32) ->
    relu+bias on ScalarE -> one expert matmul ([35,128].T @ [35,256]) ->
    PSUM->SBUF copy -> DMA out.  Memory-bound by the 32MB/core output.
"""

import os
import sys
from contextlib import ExitStack

sys.path.insert(0, "/opt/trn_rl_repo")

import numpy as np

B = 262144
NCORES = 8
J = 16
M = 12
H = 34
HP = H + 1  # fc1 output + ones row for bias folding
S_DIM = 32  # state dim
A = J * J  # 256 action logits
NEG = np.float32(-1.0e9)
TILE = 128
CHUNK = 4  # tiles per fc1 chunk (512 columns)

_BUILD_CACHE: dict = {}
LAST_RESULT = None  # BassKernelResults of the most recent run (for profiling)


def _build(T: int, expert_of_tile: tuple) -> "object":
    import concourse.bass as bass
    import concourse.tile as tile
    from concourse import bacc, mybir

    f32 = mybir.dt.float32
    nc = bacc.Bacc("TRN2", target_bir_lowering=False, debug=False)

    statesT_d = nc.declare_dram_parameter("statesT", [S_DIM, T * TILE], f32, isOutput=False)
    w1t_d = nc.declare_dram_parameter("w1t", [S_DIM, HP], f32, isOutput=False)
    b1_d = nc.declare_dram_parameter("b1", [HP, 1], f32, isOutput=False)
    weff_d = nc.declare_dram_parameter("weff", [HP, M * A], f32, isOutput=False)
    out_d = nc.declare_dram_parameter("out", [T * TILE, A], f32, isOutput=True)

    with tile.TileContext(nc) as tc, ExitStack() as ctx:
        cpool = ctx.enter_context(tc.tile_pool(name="const", bufs=1))
        stpool = ctx.enter_context(tc.tile_pool(name="st", bufs=4))
        xapool = ctx.enter_context(tc.tile_pool(name="xa", bufs=3))
        outpool = ctx.enter_context(tc.tile_pool(name="outp", bufs=6))
        psxpool = ctx.enter_context(
            tc.tile_pool(name="psx", bufs=2, space=bass.MemorySpace.PSUM)
        )
        psopool = ctx.enter_context(
            tc.tile_pool(name="pso", bufs=4, space=bass.MemorySpace.PSUM)
        )

        w1t = cpool.tile([S_DIM, HP], f32)
        nc.sync.dma_start(w1t[:], w1t_d[:])
        b1t = cpool.tile([HP, 1], f32)
        nc.sync.dma_start(b1t[:], b1_d[:])
        weff = cpool.tile([HP, M * A], f32)
        nc.sync.dma_start(weff[:], weff_d[:])

        FREE = CHUNK * TILE
        for c in range(T // CHUNK):
            st = stpool.tile([S_DIM, FREE], f32)
            nc.sync.dma_start(st[:], statesT_d[:, c * FREE : (c + 1) * FREE])

            psx = psxpool.tile([HP, FREE], f32)
            nc.tensor.matmul(psx[:], w1t[:], st[:], start=True, stop=True)

            xa = xapool.tile([HP, FREE], f32)
            nc.scalar.activation(
                xa[:], psx[:], mybir.ActivationFunctionType.Relu, bias=b1t[:]
            )

            for j in range(CHUNK):
                t = c * CHUNK + j
                e = expert_of_tile[t]
                pso = psopool.tile([TILE, A], f32)
                nc.tensor.matmul(
                    pso[:],
                    xa[:, j * TILE : (j + 1) * TILE],
                    weff[:, e * A : (e + 1) * A],
                    start=True,
                    stop=True,
                )
                ot = outpool.tile([TILE, A], f32)
                nc.vector.tensor_copy(ot[:], pso[:])
                nc.sync.dma_start(out_d[t * TILE : (t + 1) * TILE, :], ot[:])

    nc.compile()
    return nc


def kernel(states, epoch_idx, W1, b1, Wout, bout, mask):
    global LAST_RESULT
    from concourse.bass_utils import run_bass_kernel_spmd

    states = np.asarray(states, dtype=np.float32)
    epoch_idx = np.asarray(epoch_idx, dtype=np.int32)
    W1 = np.asarray(W1, dtype=np.float32)
    b1 = np.asarray(b1, dtype=np.float32)
    Wout = np.asarray(Wout, dtype=np.float32)
    bout = np.asarray(bout, dtype=np.float32)
    mask = np.asarray(mask, dtype=np.int32)

    # --- route rows: per expert, deal round-robin across cores ---
    core_idx = [[None] * M for _ in range(NCORES)]
    for m in range(M):
        idx_m = np.nonzero(epoch_idx == m)[0]
        for i in range(NCORES):
            core_idx[i][m] = idx_m[i::NCORES]
    cnt = [[len(core_idx[i][m]) for m in range(M)] for i in range(NCORES)]
    # shared per-expert tile capacity across cores
    nt = [max((cnt[i][m] + TILE - 1) // TILE for i in range(NCORES)) for m in range(M)]
    nt[-1] += (-sum(nt)) % CHUNK  # make total a multiple of CHUNK
    T = sum(nt)
    S = np.concatenate([[0], np.cumsum(nt)])  # tile offset of each expert
    expert_of_tile = []
    for m in range(M):
        expert_of_tile += [m] * nt[m]
    expert_of_tile = tuple(expert_of_tile)

    # --- fold fc1 weights (+ ones row) ---
    w1t_aug = np.zeros((S_DIM, HP), np.float32)
    w1t_aug[:, :H] = W1.T
    b1_aug = np.zeros((HP, 1), np.float32)
    b1_aug[:H, 0] = b1
    b1_aug[H, 0] = 1.0  # relu(0*x + 1) = 1 -> ones row for bias folding

    # --- effective expert weights: mask + bout folded in ---
    keep = mask.reshape(A) != 0
    weff = np.zeros((HP, M * A), np.float32)
    for m in range(M):
        weff[:H, m * A : (m + 1) * A] = np.where(keep[None, :], Wout[m].T, 0.0)
        weff[H, m * A : (m + 1) * A] = np.where(keep, bout[m], NEG)

    # --- pack per-core transposed states ---
    in_maps = []
    for i in range(NCORES):
        packed = np.zeros((S_DIM, T * TILE), np.float32)
        for m in range(M):
            r0 = S[m] * TILE
            packed[:, r0 : r0 + cnt[i][m]] = states[core_idx[i][m]].T
        in_maps.append(
            {"statesT": packed, "w1t": w1t_aug, "b1": b1_aug, "weff": weff}
        )

    key = (T, expert_of_tile)
    nc = _BUILD_CACHE.get(key)
    if nc is None:
        nc = _build(T, expert_of_tile)
        _BUILD_CACHE[key] = nc

    res = run_bass_kernel_spmd(nc, in_maps, core_ids=list(range(NCORES)))
    LAST_RESULT = res

    out_full = np.empty((B, A), np.float32)
    for i in range(NCORES):
        oc = res.results[i]["out"]
        for m in range(M):
            r0 = S[m] * TILE
            out_full[core_idx[i][m]] = oc[r0 : r0 + cnt[i][m]]
    return out_full.reshape(B, J, J)


# revision 2
# speedup vs baseline: 1.2888x; 1.2888x over previous
"""MoE-routing actor kernel for 8 Trainium2 NeuronCores.

Strategy (pure data parallel, expert-sorted, bf16 compute):
  - Host: for each expert m, deal its rows round-robin to the 8 cores so all
    cores get near-identical per-expert counts and can share ONE SPMD graph.
  - Host packs each core's rows grouped by expert (padded to 128-row tiles),
    pre-transposed to [32, T*128] bf16 so the device needs no transposes.
  - fc1 bias + expert bias are folded in via an extra all-ones activation row
    (row 34) whose expert-weight row holds bout.
  - The mask is applied on the host: the device only computes/writes the
    kept output columns; masked columns are exact -1e9 filled host-side.
  - Device per 128-row tile: fc1 matmul (stationary W1T_aug, K=32) ->
    relu+bias on ScalarE -> one expert matmul ([35,128].T @ [35,Ak]) ->
    PSUM->SBUF bf16 copy -> DMA out.
"""

import os
import sys
from contextlib import ExitStack

sys.path.insert(0, "/opt/trn_rl_repo")

import numpy as np
import ml_dtypes

BF16 = ml_dtypes.bfloat16

B = 262144
NCORES = 8
J = 16
M = 12
H = 34
HP = H + 1  # fc1 output + ones row for bias folding
S_DIM = 32  # state dim
A = J * J  # 256 action logits
NEG = np.float32(-1.0e9)
TILE = 128
CHUNK = 4  # tiles per fc1 chunk (512 columns)

_BUILD_CACHE: dict = {}
LAST_RESULT = None  # BassKernelResults of the most recent run (for profiling)


def _build(T: int, expert_of_tile: tuple, Ak: int) -> "object":
    import concourse.bass as bass
    import concourse.tile as tile
    from concourse import bacc, mybir

    f32 = mybir.dt.float32
    bf16 = mybir.dt.bfloat16
    nc = bacc.Bacc("TRN2", target_bir_lowering=False, debug=False)

    statesT_d = nc.declare_dram_parameter("statesT", [S_DIM, T * TILE], bf16, isOutput=False)
    w1t_d = nc.declare_dram_parameter("w1t", [S_DIM, HP], bf16, isOutput=False)
    b1_d = nc.declare_dram_parameter("b1", [HP, 1], f32, isOutput=False)
    weff_d = nc.declare_dram_parameter("weff", [HP, M * Ak], bf16, isOutput=False)
    out_d = nc.declare_dram_parameter("out", [T * TILE, Ak], bf16, isOutput=True)

    with tile.TileContext(nc) as tc, ExitStack() as ctx:
        cpool = ctx.enter_context(tc.tile_pool(name="const", bufs=1))
        stpool = ctx.enter_context(tc.tile_pool(name="st", bufs=4))
        xapool = ctx.enter_context(tc.tile_pool(name="xa", bufs=3))
        outpool = ctx.enter_context(tc.tile_pool(name="outp", bufs=6))
        psxpool = ctx.enter_context(
            tc.tile_pool(name="psx", bufs=2, space=bass.MemorySpace.PSUM)
        )
        psopool = ctx.enter_context(
            tc.tile_pool(name="pso", bufs=4, space=bass.MemorySpace.PSUM)
        )

        w1t = cpool.tile([S_DIM, HP], bf16)
        nc.sync.dma_start(w1t[:], w1t_d[:])
        b1t = cpool.tile([HP, 1], f32)
        nc.sync.dma_start(b1t[:], b1_d[:])
        weff = cpool.tile([HP, M * Ak], bf16)
        nc.sync.dma_start(weff[:], weff_d[:])

        FREE = CHUNK * TILE
        for c in range(T // CHUNK):
            st = stpool.tile([S_DIM, FREE], bf16)
            nc.sync.dma_start(st[:], statesT_d[:, c * FREE : (c + 1) * FREE])

            psx = psxpool.tile([HP, FREE], f32)
            nc.tensor.matmul(psx[:], w1t[:], st[:], start=True, stop=True)

            xa = xapool.tile([HP, FREE], bf16)
            nc.scalar.activation(
                xa[:], psx[:], mybir.ActivationFunctionType.Relu, bias=b1t[:]
            )

            for j in range(CHUNK):
                t = c * CHUNK + j
                e = expert_of_tile[t]
                pso = psopool.tile([TILE, Ak], f32)
                nc.tensor.matmul(
                    pso[:],
                    xa[:, j * TILE : (j + 1) * TILE],
                    weff[:, e * Ak : (e + 1) * Ak],
                    start=True,
                    stop=True,
                )
                ot = outpool.tile([TILE, Ak], bf16)
                nc.vector.tensor_copy(ot[:], pso[:])
                nc.sync.dma_start(out_d[t * TILE : (t + 1) * TILE, :], ot[:])

    nc.compile()
    return nc


def kernel(states, epoch_idx, W1, b1, Wout, bout, mask):
    global LAST_RESULT
    from concourse.bass_utils import run_bass_kernel_spmd

    states = np.asarray(states, dtype=np.float32)
    epoch_idx = np.asarray(epoch_idx, dtype=np.int32)
    W1 = np.asarray(W1, dtype=np.float32)
    b1 = np.asarray(b1, dtype=np.float32)
    Wout = np.asarray(Wout, dtype=np.float32)
    bout = np.asarray(bout, dtype=np.float32)
    mask = np.asarray(mask, dtype=np.int32)

    keep = mask.reshape(A) != 0
    kept_cols = np.nonzero(keep)[0]
    Ak = int(len(kept_cols))
    if Ak == 0:
        return np.full((B, J, J), NEG, np.float32)

    # --- route rows: per expert, deal round-robin across cores ---
    core_idx = [[None] * M for _ in range(NCORES)]
    for m in range(M):
        idx_m = np.nonzero(epoch_idx == m)[0]
        for i in range(NCORES):
            core_idx[i][m] = idx_m[i::NCORES]
    cnt = [[len(core_idx[i][m]) for m in range(M)] for i in range(NCORES)]
    # shared per-expert tile capacity across cores
    nt = [max((cnt[i][m] + TILE - 1) // TILE for i in range(NCORES)) for m in range(M)]
    nt[-1] += (-sum(nt)) % CHUNK  # make total a multiple of CHUNK
    T = sum(nt)
    S = np.concatenate([[0], np.cumsum(nt)])  # tile offset of each expert
    expert_of_tile = []
    for m in range(M):
        expert_of_tile += [m] * nt[m]
    expert_of_tile = tuple(expert_of_tile)

    # --- fold fc1 weights (+ ones row) ---
    w1t_aug = np.zeros((S_DIM, HP), np.float32)
    w1t_aug[:, :H] = W1.T
    b1_aug = np.zeros((HP, 1), np.float32)
    b1_aug[:H, 0] = b1
    b1_aug[H, 0] = 1.0  # relu(0*x + 1) = 1 -> ones row for bias folding

    # --- effective expert weights (kept columns only; bout in ones row) ---
    weff = np.zeros((HP, M * Ak), np.float32)
    for m in range(M):
        weff[:H, m * Ak : (m + 1) * Ak] = Wout[m][kept_cols].T
        weff[H, m * Ak : (m + 1) * Ak] = bout[m][kept_cols]

    # --- pack per-core transposed states (bf16) ---
    in_maps = []
    w1t_bf = w1t_aug.astype(BF16)
    weff_bf = weff.astype(BF16)
    for i in range(NCORES):
        packed = np.zeros((S_DIM, T * TILE), BF16)
        for m in range(M):
            r0 = S[m] * TILE
            packed[:, r0 : r0 + cnt[i][m]] = states[core_idx[i][m]].T.astype(BF16)
        in_maps.append(
            {"statesT": packed, "w1t": w1t_bf, "b1": b1_aug, "weff": weff_bf}
        )

    key = (T, expert_of_tile, Ak)
    nc = _BUILD_CACHE.get(key)
    if nc is None:
        nc = _build(T, expert_of_tile, Ak)
        _BUILD_CACHE[key] = nc

    res = run_bass_kernel_spmd(nc, in_maps, core_ids=list(range(NCORES)))
    LAST_RESULT = res

    out_kept = np.empty((B, Ak), np.float32)
    for i in range(NCORES):
        oc = res.results[i]["out"]
        for m in range(M):
            r0 = S[m] * TILE
            out_kept[core_idx[i][m]] = oc[r0 : r0 + cnt[i][m]]
    out_full = np.full((B, A), NEG, np.float32)
    out_full[:, kept_cols] = out_kept
    return out_full.reshape(B, J, J)


# revision 3
# speedup vs baseline: 4.4753x; 3.4724x over previous
"""MoE-routing actor kernel for 8 Trainium2 NeuronCores.

Strategy (pure data parallel, expert-sorted, bf16 compute):
  - Host: for each expert m, deal its rows round-robin to the 8 cores so all
    cores get near-identical per-expert counts and can share ONE SPMD graph.
  - Host packs each core's rows grouped by expert (padded to 128-row tiles),
    pre-transposed so the device needs no transposes.
  - fc1 bias + expert bias are folded in via an extra all-ones activation row
    (row 34) whose expert-weight row holds bout.
  - Mask applied host-side: the device computes only the first A_DEV (<=128)
    kept output columns; masked columns are exact -1e9 filled host-side and
    any kept columns beyond A_DEV (typically ~9) are computed on host BLAS.
  - Device loop per 8-tile super-chunk (1024 rows):
      gpsimd-DMA statesT [32,1024] -> 2x fc1 matmul (stationary W1T_aug) into
      [35,1024] PSUM -> 1x Relu+bias ACT -> per 4-tile group: 4 expert
      matmuls ([35,128].T @ [35,A_DEV]) into one PSUM bank -> 1 CAST to bf16
      -> 1 contiguous 128KB DMA out (sync queue).
    Separate DMA queues for loads (gpsimd) vs stores (sync) avoid
    head-of-line blocking on the in-order sequencers.
"""

import os
import sys
from contextlib import ExitStack

sys.path.insert(0, "/opt/trn_rl_repo")

import numpy as np
import ml_dtypes

BF16 = ml_dtypes.bfloat16

B = 262144
NCORES = 8
J = 16
M = 12
H = 34
HP = H + 1  # fc1 output + ones row for bias folding
S_DIM = 32  # state dim
A = J * J  # 256 action logits
NEG = np.float32(-1.0e9)
TILE = 128
GROUP = 4  # tiles per PSUM output bank / cast / store
SUPER = 8  # tiles per fc1 chunk (1024 columns)

_BUILD_CACHE: dict = {}
LAST_RESULT = None  # BassKernelResults of the most recent run (for profiling)


def _build(T: int, expert_of_tile: tuple, Adev: int) -> "object":
    import concourse.bass as bass
    import concourse.tile as tile
    from concourse import bacc, mybir

    f32 = mybir.dt.float32
    bf16 = mybir.dt.bfloat16
    nc = bacc.Bacc("TRN2", target_bir_lowering=False, debug=False)

    n_super = T // SUPER
    n_group = T // GROUP
    statesT_d = nc.declare_dram_parameter(
        "statesT", [n_super, S_DIM, SUPER * TILE], bf16, isOutput=False
    )
    w1t_d = nc.declare_dram_parameter("w1t", [S_DIM, HP], bf16, isOutput=False)
    b1_d = nc.declare_dram_parameter("b1", [HP, 1], f32, isOutput=False)
    weff_d = nc.declare_dram_parameter("weff", [HP, M * Adev], bf16, isOutput=False)
    out_d = nc.declare_dram_parameter(
        "out", [n_group, TILE, GROUP * Adev], bf16, isOutput=True
    )

    with tile.TileContext(nc) as tc, ExitStack() as ctx:
        cpool = ctx.enter_context(tc.tile_pool(name="const", bufs=1))
        stpool = ctx.enter_context(tc.tile_pool(name="st", bufs=4))
        xapool = ctx.enter_context(tc.tile_pool(name="xa", bufs=4))
        outpool = ctx.enter_context(tc.tile_pool(name="outp", bufs=6))
        psxpool = ctx.enter_context(
            tc.tile_pool(name="psx", bufs=2, space=bass.MemorySpace.PSUM)
        )
        psopool = ctx.enter_context(
            tc.tile_pool(name="pso", bufs=4, space=bass.MemorySpace.PSUM)
        )

        w1t = cpool.tile([S_DIM, HP], bf16)
        nc.gpsimd.dma_start(w1t[:], w1t_d[:])
        b1t = cpool.tile([HP, 1], f32)
        nc.gpsimd.dma_start(b1t[:], b1_d[:])
        weff = cpool.tile([HP, M * Adev], bf16)
        nc.gpsimd.dma_start(weff[:], weff_d[:])

        SFREE = SUPER * TILE  # 1024
        for sc in range(n_super):
            st = stpool.tile([S_DIM, SFREE], bf16)
            nc.gpsimd.dma_start(st[:], statesT_d[sc])

            psx = psxpool.tile([HP, SFREE], f32)
            nc.tensor.matmul(psx[:, 0:512], w1t[:], st[:, 0:512], start=True, stop=True)
            nc.tensor.matmul(
                psx[:, 512:1024], w1t[:], st[:, 512:1024], start=True, stop=True
            )

            xa = xapool.tile([HP, SFREE], bf16)
            nc.scalar.activation(
                xa[:], psx[:], mybir.ActivationFunctionType.Relu, bias=b1t[:]
            )

            for g in range(SUPER // GROUP):
                pso = psopool.tile([TILE, GROUP * Adev], f32)
                for j in range(GROUP):
                    t = sc * SUPER + g * GROUP + j
                    e = expert_of_tile[t]
                    nc.tensor.matmul(
                        pso[:, j * Adev : (j + 1) * Adev],
                        xa[:, (g * GROUP + j) * TILE : (g * GROUP + j + 1) * TILE],
                        weff[:, e * Adev : (e + 1) * Adev],
                        start=True,
                        stop=True,
                    )
                ot = outpool.tile([TILE, GROUP * Adev], bf16)
                nc.vector.tensor_copy(ot[:], pso[:])
                nc.sync.dma_start(out_d[sc * (SUPER // GROUP) + g], ot[:])

    nc.compile()
    return nc


def kernel(states, epoch_idx, W1, b1, Wout, bout, mask):
    global LAST_RESULT
    from concourse.bass_utils import run_bass_kernel_spmd

    states = np.asarray(states, dtype=np.float32)
    epoch_idx = np.asarray(epoch_idx, dtype=np.int32)
    W1 = np.asarray(W1, dtype=np.float32)
    b1 = np.asarray(b1, dtype=np.float32)
    Wout = np.asarray(Wout, dtype=np.float32)
    bout = np.asarray(bout, dtype=np.float32)
    mask = np.asarray(mask, dtype=np.int32)

    keep = mask.reshape(A) != 0
    kept_cols = np.nonzero(keep)[0]
    Ak = int(len(kept_cols))
    if Ak == 0:
        return np.full((B, J, J), NEG, np.float32)
    Adev = min(Ak, TILE)
    dev_cols = kept_cols[:Adev]
    rem_cols = kept_cols[Adev:]

    # --- route rows: per expert, deal round-robin across cores ---
    core_idx = [[None] * M for _ in range(NCORES)]
    for m in range(M):
        idx_m = np.nonzero(epoch_idx == m)[0]
        for i in range(NCORES):
            core_idx[i][m] = idx_m[i::NCORES]
    cnt = [[len(core_idx[i][m]) for m in range(M)] for i in range(NCORES)]
    # shared per-expert tile capacity across cores
    nt = [max((cnt[i][m] + TILE - 1) // TILE for i in range(NCORES)) for m in range(M)]
    nt[-1] += (-sum(nt)) % SUPER  # make total a multiple of SUPER
    T = sum(nt)
    S = np.concatenate([[0], np.cumsum(nt)])  # tile offset of each expert
    expert_of_tile = []
    for m in range(M):
        expert_of_tile += [m] * nt[m]
    expert_of_tile = tuple(expert_of_tile)

    # --- fold fc1 weights (+ ones row) ---
    w1t_aug = np.zeros((S_DIM, HP), np.float32)
    w1t_aug[:, :H] = W1.T
    b1_aug = np.zeros((HP, 1), np.float32)
    b1_aug[:H, 0] = b1
    b1_aug[H, 0] = 1.0  # relu(0*x + 1) = 1 -> ones row for bias folding

    # --- effective expert weights (device columns only; bout in ones row) ---
    weff = np.zeros((HP, M * Adev), np.float32)
    for m in range(M):
        weff[:H, m * Adev : (m + 1) * Adev] = Wout[m][dev_cols].T
        weff[H, m * Adev : (m + 1) * Adev] = bout[m][dev_cols]

    # --- pack per-core transposed states (bf16, super-chunk-major) ---
    in_maps = []
    w1t_bf = w1t_aug.astype(BF16)
    weff_bf = weff.astype(BF16)
    for i in range(NCORES):
        packed = np.zeros((T * TILE, S_DIM), np.float32)
        for m in range(M):
            r0 = S[m] * TILE
            packed[r0 : r0 + cnt[i][m]] = states[core_idx[i][m]]
        sc_major = np.ascontiguousarray(
            packed.astype(BF16).reshape(T // SUPER, SUPER * TILE, S_DIM).transpose(0, 2, 1)
        )
        in_maps.append(
            {"statesT": sc_major, "w1t": w1t_bf, "b1": b1_aug, "weff": weff_bf}
        )

    key = (T, expert_of_tile, Adev)
    nc = _BUILD_CACHE.get(key)
    if nc is None:
        nc = _build(T, expert_of_tile, Adev)
        _BUILD_CACHE[key] = nc

    res = run_bass_kernel_spmd(nc, in_maps, core_ids=list(range(NCORES)))
    LAST_RESULT = res

    # --- unpack: group-major [T/4, 128, 4*Adev] -> rows [T*128, Adev] ---
    out_kept = np.empty((B, Adev), np.float32)
    for i in range(NCORES):
        oc = np.asarray(res.results[i]["out"])
        rows = (
            oc.reshape(T // GROUP, TILE, GROUP, Adev)
            .transpose(0, 2, 1, 3)
            .reshape(T * TILE, Adev)
            .astype(np.float32)
        )
        for m in range(M):
            r0 = S[m] * TILE
            out_kept[core_idx[i][m]] = rows[r0 : r0 + cnt[i][m]]

    out_full = np.full((B, A), NEG, np.float32)
    out_full[:, dev_cols] = out_kept

    # --- host remainder: kept columns beyond the device's 128 ---
    if len(rem_cols):
        x = np.maximum(states @ W1.T + b1[None, :], 0.0)  # [B, H]
        for m in range(M):
            rows_m = np.nonzero(epoch_idx == m)[0]
            out_full[rows_m[:, None], rem_cols[None, :]] = (
                x[rows_m] @ Wout[m][rem_cols].T + bout[m][rem_cols][None, :]
            )

    return out_full.reshape(B, J, J)


# revision 4
# speedup vs baseline: 4.5579x; 1.0184x over previous
"""MoE-routing actor kernel for 8 Trainium2 NeuronCores.

Strategy (pure data parallel, expert-sorted, bf16 compute):
  - Host: for each expert m, deal its rows round-robin to the 8 cores so all
    cores get near-identical per-expert counts and can share ONE SPMD graph.
    Per-expert row capacities are the max count over cores (row-granular,
    ~0.3% padding); rows are packed sorted-by-expert, pre-transposed.
  - fc1 bias + expert bias folded in via an extra all-ones activation row
    (row 34) whose expert-weight row holds bout.
  - Mask applied host-side: the device computes only the first A_DEV (<=128)
    kept output columns; masked columns are exact -1e9 filled host-side and
    kept columns beyond A_DEV (typically ~9) are computed on host BLAS.
  - Device, per 1024-row super-chunk:
      statesT [32,1024] bf16 load (gpsimd queue) with operands parked on
      partitions 96..127 -> 2 fc1 matmuls on PE row-strip 3
      (tile_position=(96,0)) running on sub-arrays disjoint from the expert
      matmuls -> Relu+bias ACT -> transposed expert matmuls (stationary
      weff_e [35,128], moving xa run of <=512 rows, boundary runs split) into
      a [128,1024] PSUM pair -> one [128,1024] f32->bf16 DVE cast -> one
      256KB contiguous store (sync queue).
    Emission is software-pipelined (fc1 of super s before expert phase of
    s-1) so ACT/DVE/PE/DMA all stream concurrently.
"""

import os
import sys
from contextlib import ExitStack

sys.path.insert(0, "/opt/trn_rl_repo")

import numpy as np
import ml_dtypes

BF16 = ml_dtypes.bfloat16

B = 262144
NCORES = 8
J = 16
M = 12
H = 34
HP = H + 1  # fc1 output + ones row for bias folding
S_DIM = 32  # state dim
A = J * J  # 256 action logits
NEG = np.float32(-1.0e9)
SUPER = 1024  # rows per fc1 chunk / output store
HALF = 512  # PSUM-bank / matmul free-dim granule

_BUILD_CACHE: dict = {}
LAST_RESULT = None  # BassKernelResults of the most recent run (for profiling)


def _make_runs(caps, R):
    """Per 512-row half-chunk, the (expert, col0, col1) runs covering it."""
    offs = np.concatenate([[0], np.cumsum(caps)])
    assert offs[-1] == R
    runs = [[] for _ in range(R // HALF)]
    for m in range(len(caps)):
        lo, hi = int(offs[m]), int(offs[m + 1])
        g0, g1 = lo // HALF, (hi - 1) // HALF
        for g in range(g0, g1 + 1):
            a = max(lo, g * HALF)
            b = min(hi, (g + 1) * HALF)
            if a < b:
                runs[g].append((m, a, b))
    return runs


def _build(R: int, caps: tuple, Adev: int) -> "object":
    import concourse.bass as bass
    import concourse.tile as tile
    from concourse import bacc, mybir

    f32 = mybir.dt.float32
    bf16 = mybir.dt.bfloat16
    nc = bacc.Bacc("TRN2", target_bir_lowering=False, debug=False)

    n_super = R // SUPER
    runs = _make_runs(list(caps), R)

    statesT_d = nc.declare_dram_parameter(
        "statesT", [n_super, S_DIM, SUPER], bf16, isOutput=False
    )
    w1t_d = nc.declare_dram_parameter("w1t", [S_DIM, HP], bf16, isOutput=False)
    b1_d = nc.declare_dram_parameter("b1", [HP, 1], f32, isOutput=False)
    weff_d = nc.declare_dram_parameter("weff", [HP, M * Adev], bf16, isOutput=False)
    out_d = nc.declare_dram_parameter(
        "out", [n_super, Adev, SUPER], bf16, isOutput=True
    )

    with tile.TileContext(nc) as tc, ExitStack() as ctx:
        cpool = ctx.enter_context(tc.tile_pool(name="const", bufs=1))
        stpool = ctx.enter_context(tc.tile_pool(name="st", bufs=4))
        xapool = ctx.enter_context(tc.tile_pool(name="xa", bufs=3))
        outpool = ctx.enter_context(tc.tile_pool(name="outp", bufs=4))
        psxpool = ctx.enter_context(
            tc.tile_pool(name="psx", bufs=2, space=bass.MemorySpace.PSUM)
        )
        psopool = ctx.enter_context(
            tc.tile_pool(name="pso", bufs=2, space=bass.MemorySpace.PSUM)
        )

        # fc1 operands parked on partitions 96..127 (PE row-strip 3) so fc1
        # matmuls use sub-arrays disjoint from the expert matmuls' strips 0-1.
        w1t = cpool.tile([128, HP], bf16)
        nc.gpsimd.dma_start(w1t[96 : 96 + S_DIM, :], w1t_d[:])
        b1t = cpool.tile([HP, 1], f32)
        nc.gpsimd.dma_start(b1t[:], b1_d[:])
        weff = cpool.tile([HP, M * Adev], bf16)
        nc.gpsimd.dma_start(weff[:], weff_d[:])

        xas = [None] * n_super
        psos = [None] * n_super

        for sc in range(n_super + 1):
            if sc < n_super:
                st = stpool.tile([128, SUPER], bf16)
                nc.gpsimd.dma_start(st[96 : 96 + S_DIM, :], statesT_d[sc])
                psx = psxpool.tile([HP, SUPER], f32)
                for h in range(2):
                    nc.tensor.matmul(
                        psx[:, h * HALF : (h + 1) * HALF],
                        w1t[96 : 96 + S_DIM, :],
                        st[96 : 96 + S_DIM, h * HALF : (h + 1) * HALF],
                        start=True,
                        stop=True,
                        tile_position=(96, 0),
                    )
                xa = xapool.tile([HP, SUPER], bf16)
                nc.scalar.activation(
                    xa[:], psx[:], mybir.ActivationFunctionType.Relu, bias=b1t[:]
                )
                xas[sc] = xa

            if sc >= 1:
                p = sc - 1
                pso = psopool.tile([Adev, SUPER], f32)
                for h in range(2):
                    g = p * 2 + h
                    for (m, a, b) in runs[g]:
                        c0 = a - p * SUPER
                        c1 = b - p * SUPER
                        nc.tensor.matmul(
                            pso[:, c0:c1],
                            weff[:, m * Adev : (m + 1) * Adev],
                            xas[p][:, c0:c1],
                            start=True,
                            stop=True,
                        )
                ot = outpool.tile([Adev, SUPER], bf16)
                nc.vector.tensor_copy(ot[:], pso[:])
                nc.sync.dma_start(out_d[p], ot[:])

    nc.compile()
    return nc


def kernel(states, epoch_idx, W1, b1, Wout, bout, mask):
    global LAST_RESULT
    from concourse.bass_utils import run_bass_kernel_spmd

    states = np.asarray(states, dtype=np.float32)
    epoch_idx = np.asarray(epoch_idx, dtype=np.int32)
    W1 = np.asarray(W1, dtype=np.float32)
    b1 = np.asarray(b1, dtype=np.float32)
    Wout = np.asarray(Wout, dtype=np.float32)
    bout = np.asarray(bout, dtype=np.float32)
    mask = np.asarray(mask, dtype=np.int32)

    keep = mask.reshape(A) != 0
    kept_cols = np.nonzero(keep)[0]
    Ak = int(len(kept_cols))
    if Ak == 0:
        return np.full((B, J, J), NEG, np.float32)
    Adev = min(Ak, 128)
    dev_cols = kept_cols[:Adev]
    rem_cols = kept_cols[Adev:]

    # --- route rows: per expert, deal round-robin across cores ---
    core_idx = [[None] * M for _ in range(NCORES)]
    for m in range(M):
        idx_m = np.nonzero(epoch_idx == m)[0]
        for i in range(NCORES):
            core_idx[i][m] = idx_m[i::NCORES]
    cnt = [[len(core_idx[i][m]) for m in range(M)] for i in range(NCORES)]
    # shared per-expert row capacity across cores (row-granular)
    caps = [max(cnt[i][m] for i in range(NCORES)) for m in range(M)]
    need = sum(caps)
    R = SUPER * ((max(need, B // NCORES) + SUPER - 1) // SUPER)
    caps[-1] += R - need  # dump slack into the last expert
    caps = tuple(caps)
    offs = np.concatenate([[0], np.cumsum(caps)])

    # --- fold fc1 weights (+ ones row) ---
    w1t_aug = np.zeros((S_DIM, HP), np.float32)
    w1t_aug[:, :H] = W1.T
    b1_aug = np.zeros((HP, 1), np.float32)
    b1_aug[:H, 0] = b1
    b1_aug[H, 0] = 1.0  # relu(0*x + 1) = 1 -> ones row for bias folding

    # --- effective expert weights (device columns only; bout in ones row) ---
    weff = np.zeros((HP, M * Adev), np.float32)
    for m in range(M):
        weff[:H, m * Adev : (m + 1) * Adev] = Wout[m][dev_cols].T
        weff[H, m * Adev : (m + 1) * Adev] = bout[m][dev_cols]

    # --- pack per-core transposed states (bf16, super-chunk-major) ---
    in_maps = []
    w1t_bf = w1t_aug.astype(BF16)
    weff_bf = weff.astype(BF16)
    for i in range(NCORES):
        packed = np.zeros((R, S_DIM), np.float32)
        for m in range(M):
            r0 = int(offs[m])
            packed[r0 : r0 + cnt[i][m]] = states[core_idx[i][m]]
        sc_major = np.ascontiguousarray(
            packed.astype(BF16).reshape(R // SUPER, SUPER, S_DIM).transpose(0, 2, 1)
        )
        in_maps.append(
            {"statesT": sc_major, "w1t": w1t_bf, "b1": b1_aug, "weff": weff_bf}
        )

    key = (R, caps, Adev)
    nc = _BUILD_CACHE.get(key)
    if nc is None:
        nc = _build(R, caps, Adev)
        _BUILD_CACHE[key] = nc

    res = run_bass_kernel_spmd(nc, in_maps, core_ids=list(range(NCORES)))
    LAST_RESULT = res

    # --- unpack: [n_super, Adev, 1024] -> rows [R, Adev] ---
    out_kept = np.empty((B, Adev), np.float32)
    for i in range(NCORES):
        oc = np.asarray(res.results[i]["out"])
        rows = (
            oc.transpose(0, 2, 1).reshape(R, Adev).astype(np.float32)
        )
        for m in range(M):
            r0 = int(offs[m])
            out_kept[core_idx[i][m]] = rows[r0 : r0 + cnt[i][m]]

    out_full = np.full((B, A), NEG, np.float32)
    out_full[:, dev_cols] = out_kept

    # --- host remainder: kept columns beyond the device's 128 ---
    if len(rem_cols):
        x = np.maximum(states @ W1.T + b1[None, :], 0.0)  # [B, H]
        for m in range(M):
            rows_m = np.nonzero(epoch_idx == m)[0]
            out_full[rows_m[:, None], rem_cols[None, :]] = (
                x[rows_m] @ Wout[m][rem_cols].T + bout[m][rem_cols][None, :]
            )

    return out_full.reshape(B, J, J)


# revision 5
# speedup vs baseline: 6.2136x; 1.3633x over previous
"""MoE-routing actor kernel for 8 Trainium2 NeuronCores.

Strategy (pure data parallel, expert-sorted, bf16 compute):
  - Host: for each expert m, deal its rows round-robin to the 8 cores so all
    cores get near-identical per-expert counts and can share ONE SPMD graph.
    Per-expert row capacities are the max count over cores (row-granular,
    ~0.3% padding); rows are packed sorted-by-expert.
  - The tiny shared trunk (fc1: 262144x32 @ 32x34, 0.6 GFLOP) plus relu runs
    on host BLAS; the device gets pre-packed transposed activations
    xaT [35, R] bf16 with an all-ones row 34 that folds the expert bias bout
    into the expert matmul.
  - Mask applied host-side: the device computes only the first A_DEV (<=128)
    kept output columns; masked columns are exact -1e9 filled host-side and
    kept columns beyond A_DEV (typically ~9) are computed on host BLAS.
  - Device, per 1024-row super-chunk: xaT [35,1024] load (gpsimd queue) ->
    transposed expert matmuls (stationary weff_e [35,A_DEV], moving xa run of
    <=512 rows, expert-boundary runs split) into a [A_DEV,1024] PSUM pair ->
    one [A_DEV,1024] f32->bf16 cast, alternating VectorE/ScalarE ->
    one 256KB contiguous store (sync queue).
  The device work is a single dense GEMM stream: ~1 PE cycle/row at the
  fixed 1.2 GHz PE clock, overlapped with casts and DMA.
"""

import os
import sys
from contextlib import ExitStack

sys.path.insert(0, "/opt/trn_rl_repo")

import numpy as np
import ml_dtypes

BF16 = ml_dtypes.bfloat16

B = 262144
NCORES = 8
J = 16
M = 12
H = 34
HP = H + 1  # fc1 output + ones row for bias folding
S_DIM = 32  # state dim
A = J * J  # 256 action logits
NEG = np.float32(-1.0e9)
SUPER = 1024  # rows per load/store chunk
HALF = 512  # PSUM-bank / matmul free-dim granule

_BUILD_CACHE: dict = {}
LAST_RESULT = None  # BassKernelResults of the most recent run (for profiling)


def _make_runs(caps, R):
    """Per 512-row half-chunk, the (expert, row0, row1) runs covering it."""
    offs = np.concatenate([[0], np.cumsum(caps)])
    assert offs[-1] == R
    runs = [[] for _ in range(R // HALF)]
    for m in range(len(caps)):
        lo, hi = int(offs[m]), int(offs[m + 1])
        if lo >= hi:
            continue
        for g in range(lo // HALF, (hi - 1) // HALF + 1):
            a = max(lo, g * HALF)
            b = min(hi, (g + 1) * HALF)
            if a < b:
                runs[g].append((m, a, b))
    return runs


def _build(R: int, caps: tuple, Adev: int) -> "object":
    import concourse.bass as bass
    import concourse.tile as tile
    from concourse import bacc, mybir

    f32 = mybir.dt.float32
    bf16 = mybir.dt.bfloat16
    nc = bacc.Bacc("TRN2", target_bir_lowering=False, debug=False)

    n_super = R // SUPER
    runs = _make_runs(list(caps), R)

    xat_d = nc.declare_dram_parameter("xat", [n_super, HP, SUPER], bf16, isOutput=False)
    weff_d = nc.declare_dram_parameter("weff", [HP, M * Adev], bf16, isOutput=False)
    out_d = nc.declare_dram_parameter(
        "out", [n_super, Adev, SUPER], bf16, isOutput=True
    )

    with tile.TileContext(nc) as tc, ExitStack() as ctx:
        cpool = ctx.enter_context(tc.tile_pool(name="const", bufs=1))
        xapool = ctx.enter_context(tc.tile_pool(name="xa", bufs=4))
        outpool = ctx.enter_context(tc.tile_pool(name="outp", bufs=4))
        psopool = ctx.enter_context(
            tc.tile_pool(name="pso", bufs=3, space=bass.MemorySpace.PSUM)
        )

        weff = cpool.tile([HP, M * Adev], bf16)
        nc.gpsimd.dma_start(weff[:], weff_d[:])

        for sc in range(n_super):
            xa = xapool.tile([HP, SUPER], bf16)
            nc.gpsimd.dma_start(xa[:], xat_d[sc])

            pso = psopool.tile([Adev, SUPER], f32)
            for h in range(2):
                for (m, a, b) in runs[sc * 2 + h]:
                    c0 = a - sc * SUPER
                    c1 = b - sc * SUPER
                    nc.tensor.matmul(
                        pso[:, c0:c1],
                        weff[:, m * Adev : (m + 1) * Adev],
                        xa[:, c0:c1],
                        start=True,
                        stop=True,
                    )

            ot = outpool.tile([Adev, SUPER], bf16)
            if sc % 2 == 0:
                nc.vector.tensor_copy(ot[:], pso[:])
            else:
                nc.scalar.copy(ot[:], pso[:])
            nc.sync.dma_start(out_d[sc], ot[:])

    nc.compile()
    return nc


def kernel(states, epoch_idx, W1, b1, Wout, bout, mask):
    global LAST_RESULT
    from concourse.bass_utils import run_bass_kernel_spmd

    states = np.asarray(states, dtype=np.float32)
    epoch_idx = np.asarray(epoch_idx, dtype=np.int32)
    W1 = np.asarray(W1, dtype=np.float32)
    b1 = np.asarray(b1, dtype=np.float32)
    Wout = np.asarray(Wout, dtype=np.float32)
    bout = np.asarray(bout, dtype=np.float32)
    mask = np.asarray(mask, dtype=np.int32)

    keep = mask.reshape(A) != 0
    kept_cols = np.nonzero(keep)[0]
    Ak = int(len(kept_cols))
    if Ak == 0:
        return np.full((B, J, J), NEG, np.float32)
    Adev = min(Ak, 128)
    dev_cols = kept_cols[:Adev]
    rem_cols = kept_cols[Adev:]

    # --- shared trunk on host (tiny: ~0.6 GFLOP BLAS) ---
    x = np.maximum(states @ W1.T + b1[None, :], 0.0)  # [B, H] f32

    # --- route rows: per expert, deal round-robin across cores ---
    core_idx = [[None] * M for _ in range(NCORES)]
    for m in range(M):
        idx_m = np.nonzero(epoch_idx == m)[0]
        for i in range(NCORES):
            core_idx[i][m] = idx_m[i::NCORES]
    cnt = [[len(core_idx[i][m]) for m in range(M)] for i in range(NCORES)]
    # shared per-expert row capacity across cores (row-granular)
    caps = [max(cnt[i][m] for i in range(NCORES)) for m in range(M)]
    need = sum(caps)
    R = SUPER * ((max(need, B // NCORES) + SUPER - 1) // SUPER)
    caps[-1] += R - need  # dump slack into the last expert
    caps = tuple(caps)
    offs = np.concatenate([[0], np.cumsum(caps)])

    # --- effective expert weights (device columns only; bout in ones row) ---
    weff = np.zeros((HP, M * Adev), np.float32)
    for m in range(M):
        weff[:H, m * Adev : (m + 1) * Adev] = Wout[m][dev_cols].T
        weff[H, m * Adev : (m + 1) * Adev] = bout[m][dev_cols]
    weff_bf = weff.astype(BF16)

    # --- pack per-core transposed activations (bf16, super-chunk-major) ---
    in_maps = []
    for i in range(NCORES):
        packed = np.zeros((R, HP), np.float32)
        packed[:, H] = 1.0  # ones row for bias folding
        for m in range(M):
            r0 = int(offs[m])
            packed[r0 : r0 + cnt[i][m], :H] = x[core_idx[i][m]]
        xat = np.ascontiguousarray(
            packed.astype(BF16).reshape(R // SUPER, SUPER, HP).transpose(0, 2, 1)
        )
        in_maps.append({"xat": xat, "weff": weff_bf})

    key = (R, caps, Adev)
    nc = _BUILD_CACHE.get(key)
    if nc is None:
        nc = _build(R, caps, Adev)
        _BUILD_CACHE[key] = nc

    res = run_bass_kernel_spmd(nc, in_maps, core_ids=list(range(NCORES)))
    LAST_RESULT = res

    # --- unpack: [n_super, Adev, 1024] -> rows [R, Adev] ---
    out_kept = np.empty((B, Adev), np.float32)
    for i in range(NCORES):
        oc = np.asarray(res.results[i]["out"])
        rows = oc.transpose(0, 2, 1).reshape(R, Adev).astype(np.float32)
        for m in range(M):
            r0 = int(offs[m])
            out_kept[core_idx[i][m]] = rows[r0 : r0 + cnt[i][m]]

    out_full = np.full((B, A), NEG, np.float32)
    out_full[:, dev_cols] = out_kept

    # --- host remainder: kept columns beyond the device's 128 ---
    if len(rem_cols):
        for m in range(M):
            rows_m = np.nonzero(epoch_idx == m)[0]
            out_full[rows_m[:, None], rem_cols[None, :]] = (
                x[rows_m] @ Wout[m][rem_cols].T + bout[m][rem_cols][None, :]
            )

    return out_full.reshape(B, J, J)
